# revision 63
# baseline (speedup 1.0000x reference)
"""Multi-head attention (B=4, S=2048, D=512, H=8) on 8 TRN2 NeuronCores.

Sharding: core c handles batch b = c//2 and head-half hh = c%2 (4 of the 8
heads), with ALL 2048 queries and keys. Attention is independent per
(batch, head); the output projection is computed as a partial product
out_partial = y[:, heads_hh] @ W_out[rows_hh] and the two cores of a batch
pair are summed on the host. This removes the duplicated K/V projections the
query-split sharding required.

Per-core dataflow (bf16 inputs, fp32 PSUM accumulation):
  1. qT/kT = W^T x^T head-major [128, 2, 2048] bf16; v_aug [128 s, blk, h, 65]
     bf16 with a ones column (PV emits the softmax denominator for free).
     Projections are built from [128, 2, 512] PSUM units drawn from the SAME
     ring as the attention score tiles, so all but the first few units
     interleave into early attention slots instead of a serial phase A.
  2. Slots (qc, h, blk): scores st[128 k, 1024 q] -> exp: ACT activation
     (Exp, scale 1/8, bf16 out) for 10 of 16 slots; GPS_PAT16 slots go to
     GPSIMD as pT = (e^{1/8}) ** st via tensor_tensor(pow) (exact; GPSIMD
     cannot read PSUM so the DVE first copies st to SBUF). Splitting exp
     across two engines breaks the single-ACT softmax bottleneck; the
     pattern keeps GPS copies away from the head-boundary normalize window.
     PSUM split 2/1/1 (ACT ring / GPS ring / pv) is strongly optimal --
     sharing rings couples the exp streams and costs 15-55us.
  3. PV natural orientation: pv[128 q, 65] += pT_chunk.T @ v_aug_blk (bf16,
     free dim 65) -- half the PE cost of the transposed orientation. On hw,
     matmul start=True clears has_written for the WHOLE psum bank, so only
     the first sub-chunk matmul per bank sets it. Normalize fuses into the
     PSUM->SBUF copy (y = pv * recip(denom), per-partition broadcast), done
     per psum-bank half so the next head's PV reclaims bank 0 early.
  4. y -> yT: ONE dma_start_transpose per (qc, head-pair) ([128,1024] bf16,
     64 xbar tiles ~ 0.9us on the DMA engines); the final pair instead uses
     PE transposes emitted after the ungated qc0 outproj chunks.
  5. out_partial = yT.T @ W_out_half -> bf16 -> DMA out (4-chunk groups,
     final chunks as singles to shorten the drain). Host sums pairs.
"""

import numpy as np

import concourse.bass as bass
from concourse import bacc
import concourse.mybir as mybir
import concourse.tile as tile
from concourse.bass_utils import run_bass_kernel_spmd

B, S, D, H = 4, 2048, 512, 8
DH = D // H          # 64
P = 128
NCORES = 8
NH = 4               # heads per core
NKB = S // P         # 16 key blocks
NQC = 2              # query chunks of 1024
QC = S // NQC        # 1024
NSUB = QC // P       # 8 query sub-chunks per qc
VW = DH + 1          # 65
F32 = mybir.dt.float32
BF16 = mybir.dt.bfloat16
EXP = mybir.ActivationFunctionType.Exp
POW = mybir.AluOpType.pow
MULT = mybir.AluOpType.mult
ADD = mybir.AluOpType.add
SCALE = 1.0 / np.sqrt(DH)  # 0.125

GPS_START = 0        # GPSIMD exp engaged from the first slot
GPS_END = 128        # slots >= this: ACT only (drain the pow chain early)
GPS_PAT16 = (1, 3, 5, 10, 13, 15)  # i %% 16 in pattern -> GPSIMD exp
DELAY = 12           # PV trails the exp stream by this many slots
PT_BUFS = 13
STF_BUFS = 4
FEED_EVERY = 1
UNIT_ACT = 1000000  # every Nth unit copy goes to ACT
UNITS_ON_STG = False
FEED_VARIANT = "a"
OB_DVE_PARITY = 1
PE_WARM = 0
STG_MODE = 1  # 1: st2+stg1+pv1, 0: shared st2+pv2, 2: st3+pv1
ACT_ON_STG = ()  # ACT slots (mod 16) that borrow the stg ring
Q11_AT = 40  # slot at which the last Q unit is fed
Q01_AT = 11
PAD_AFTER = -1  # insert one empty feeder slot after step N
SPLIT_LAST_OB = False


def _build_mha(tc, out_d, xqT_d, xkT_d, xvT_d, wq_d, wk_d, wv_d, wo_d, ident_d):
    nc = tc.nc

    with (
        tc.tile_pool(name="consts", bufs=1) as cpool,
        tc.tile_pool(name="big", bufs=1) as bpool,
        tc.tile_pool(name="work", bufs=2) as wpool,
    ):
        # W_k / W_q ride the fast HWDGE queues first (they gate the first
        # projections); W_v / W_out go via the otherwise-idle SWDGE queues.
        wq_sb = cpool.tile([P, 4, 2 * P], BF16)
        wk_sb = cpool.tile([P, 4, 2 * P], BF16)
        wv_sb = cpool.tile([P, 4, 2 * P], BF16)
        wo_sb = cpool.tile([P, 2, D], BF16)
        nc.sync.dma_start(wk_sb, wk_d.rearrange("(c p) n -> p c n", p=P))
        nc.scalar.dma_start(wq_sb, wq_d.rearrange("(c p) n -> p c n", p=P))
        ident = cpool.tile([P, P], BF16)
        nc.gpsimd.dma_start(ident, ident_d)
        nc.gpsimd.dma_start(wv_sb, wv_d.rearrange("(c p) n -> p c n", p=P))
        nc.gpsimd.dma_start(wo_sb, wo_d.rearrange("(c p) n -> p c n", p=P))

        # constant e**(1/8): base for the GPSIMD pow-exp
        cbase = cpool.tile([P, 1], F32)
        nc.vector.tensor_scalar(
            out=cbase, in0=ident[:, 0:1], scalar1=0.0,
            scalar2=float(np.exp(SCALE)), op0=MULT, op1=ADD,
        )

        def load_xT(xT_d, n, name="xt", eng=None):
            t = wpool.tile([P, 4, 512], BF16, tag="xT", bufs=12, name=name)
            src = xT_d.rearrange("(c p) n -> p c n", p=P)[:, :, n * 512:(n + 1) * 512]
            (eng or nc.sync).dma_start(t, src)
            return t

        qT = bpool.tile([P, 2, S], BF16)     # [dout%128, dout//128, q]
        kT = bpool.tile([P, 2, S], BF16)
        v_aug = bpool.tile([P, NKB, NH, VW], BF16)
        yT = bpool.tile([P, 2, S], BF16)     # [d%128, d//128, q]
        y_qc = [bpool.tile([P, 2, NSUB, P], BF16, name=f"y{qc}")
                for qc in range(NQC)]        # [q%128, hpair, qsub, (h%2)*64+dh]

        nc.vector.tensor_scalar(
            out=v_aug[:, :, :, DH:VW], in0=ident[:, 0:1].broadcast_to([P, NKB, NH, 1]),
            scalar1=0.0, scalar2=1.0, op0=MULT, op1=ADD,
        )

        # Dummy exp pulls the ACT exp-table load to t=0.
        warm = cpool.tile([P, 1], F32)
        nc.scalar.activation(warm, cbase, EXP)

        with (
            tc.tile_pool(name="ps_sh", bufs=1, space="PSUM") as ps_sh,
            tc.tile_pool(name="ps_pv", bufs=1, space="PSUM") as ps_pv,
        ):
            def st_tile(name="st"):
                nb = 3 if STG_MODE == 2 else 2
                return ps_sh.tile([P, 2, 512], F32, tag="st", bufs=nb, name=name)

            def stg_tile():
                if STG_MODE == 1:
                    return ps_sh.tile([P, 2, 512], F32, tag="stg", bufs=1, name="stg")
                return st_tile()

            xk = [None] * 4
            xq = [None] * 4
            xv = [None] * 4
            xk[0] = load_xT(xkT_d, 0, name="xt_k")
            xk[1] = load_xT(xkT_d, 1, name="xt_k", eng=nc.scalar)

            # PE warm-up: ~20 throwaway ident x ident matmuls ramp the PE
            # p-state (3us of continuous busy) while the first x/W DMAs land,
            # so the real projections run at full clock from the start.
            if PE_WARM:
                wrm = ps_sh.tile([P, 2, 512], F32, tag="st", bufs=2, name="wrm")
                for t in range(PE_WARM):
                    nc.tensor.matmul(
                        wrm[:, 0, 0:P], ident, ident,
                        start=(t == 0), stop=(t == PE_WARM - 1),
                    )

            unit_rr = [0]

            def unit_copy(dst_ap, src_ap):
                unit_rr[0] += 1
                if unit_rr[0] % UNIT_ACT == 0:
                    nc.scalar.copy(dst_ap, src_ap)
                else:
                    nc.vector.tensor_copy(dst_ap, src_ap)

            def proj_unit(w_sb, dst, mc, nn, xts):
                pp = stg_tile() if UNITS_ON_STG else st_tile("pp_t")
                for j in range(2):
                    xt = xts[2 * nn + j]
                    for c in range(4):
                        nc.tensor.matmul(
                            pp[:, j, :], w_sb[:, c, mc * P:(mc + 1) * P],
                            xt[:, c, :], start=(c == 0), stop=(c == 3),
                        )
                unit_copy(
                    dst[:, mc, nn * 1024:(nn + 1) * 1024],
                    pp.rearrange("p a b -> p (a b)"),
                )

            def v_unit(k):
                # value blocks 2k, 2k+1
                if xv[k // 2] is None:
                    xv[k // 2] = load_xT(xvT_d, k // 2, name="xt_v")
                pp = stg_tile() if UNITS_ON_STG else st_tile("pp_v")
                for j in range(2):
                    blk = 2 * k + j
                    n, sb = blk // 4, blk % 4
                    for c in range(4):
                        nc.tensor.matmul(
                            pp[:, j, 0:2 * P],
                            xv[n][:, c, sb * P:(sb + 1) * P],
                            wv_sb[:, c, :], start=(c == 0), stop=(c == 3),
                        )
                unit_copy(
                    v_aug[:, 2 * k:2 * k + 2, :, 0:DH],
                    pp[:, :, 0:2 * P].rearrange("p j (h e) -> p j h e", e=DH),
                )

            # ---- minimal phase A: first halves of kT mc0 / qT mc0 ----
            # scores (qc0, h0, blk<8) only need key columns 0:1024 (nn0).
            xq[0] = load_xT(xqT_d, 0, name="xt_q")
            xq[1] = load_xT(xqT_d, 1, name="xt_q", eng=nc.scalar)
            proj_unit(wk_sb, kT, 0, 0, xk)
            proj_unit(wq_sb, qT, 0, 0, xq)

            # remaining projection units, fed one per early attention slot
            def feed_gen():
                xk[2] = load_xT(xkT_d, 2, name="xt_k")
                xk[3] = load_xT(xkT_d, 3, name="xt_k")
                proj_unit(wk_sb, kT, 0, 1, xk); yield
                if PAD_AFTER == 0: yield
                v_unit(0); yield
                if PAD_AFTER == 1: yield
                v_unit(1); yield
                if PAD_AFTER == 2: yield
                proj_unit(wk_sb, kT, 1, 0, xk); yield
                if PAD_AFTER == 3: yield
                v_unit(2); yield
                if PAD_AFTER == 4: yield
                proj_unit(wk_sb, kT, 1, 1, xk); yield
                if PAD_AFTER == 5: yield
                v_unit(3); yield
                if PAD_AFTER == 6: yield
                xq[2] = load_xT(xqT_d, 2, name="xt_q")
                xq[3] = load_xT(xqT_d, 3, name="xt_q")
                proj_unit(wq_sb, qT, 1, 0, xq); yield
                if PAD_AFTER == 7: yield
                v_unit(4); yield
                if PAD_AFTER == 8: yield
                v_unit(5); yield
                if PAD_AFTER == 9: yield
                for _ in range(Q01_AT - 9):
                    yield
                proj_unit(wq_sb, qT, 0, 1, xq); yield
                v_unit(6); yield
                v_unit(7); yield
                for _ in range(max(0, Q11_AT - 13 - (Q01_AT - 9))):
                    yield
                proj_unit(wq_sb, qT, 1, 1, xq); yield

            feeder = feed_gen()

            # ---- attention ----
            seq = [(qc, h, blk) for qc in range(NQC) for h in range(NH)
                   for blk in range(NKB)]
            fifo = []
            pv_cur = [None]

            def emit_pv(i, qc, h, blk, pT):
                if blk == 0:
                    pv_cur[0] = ps_pv.tile([P, NSUB, P], F32, tag="pv",
                                           bufs=(2 if STG_MODE == 0 else 1),
                                           name="pv")
                pv = pv_cur[0]
                for s in range(NSUB):
                    # start=True clears has_written for the WHOLE psum bank on
                    # hw, so only the first matmul touching each bank may set
                    # it (4 sub-chunk slices share a 2KB bank).
                    nc.tensor.matmul(
                        pv[:, s, 0:VW],
                        pT[:, s * P:(s + 1) * P],
                        v_aug[:, blk, h, :],
                        start=(blk == 0 and s % 4 == 0),
                        stop=(blk == NKB - 1),
                    )
                if blk == NKB - 1:
                    hp, lo = h // 2, (h % 2) * DH
                    y = y_qc[qc]
                    for hf in range(2):
                        sl = slice(hf * 4, hf * 4 + 4)
                        rec = wpool.tile([P, 4, 1], F32, tag="rec", bufs=4)
                        nc.vector.reciprocal(rec, pv[:, sl, DH:VW])
                        nc.vector.tensor_tensor(
                            out=y[:, hp, sl, lo:lo + DH], in0=pv[:, sl, 0:DH],
                            in1=rec.broadcast_to([P, 4, DH]), op=MULT,
                        )
                    if h % 2 == 1 and not (qc == NQC - 1 and h == NH - 1):
                        nc.sync.dma_start_transpose(
                            yT[:, hp, qc * QC:(qc + 1) * QC].rearrange(
                                "p (s b) -> p s b", s=NSUB),
                            y[:, hp, :, :],
                        )

            for i, (qc, h, blk) in enumerate(seq):
                if i % FEED_EVERY == 0:
                    next(feeder, None)
                mc, po = h // 2, (h % 2) * DH
                kT_h = kT[po:po + DH, mc, :]
                qT_h = qT[po:po + DH, mc, qc * QC:(qc + 1) * QC]
                is_gps = GPS_START <= i < GPS_END and i % 16 in GPS_PAT16
                st = stg_tile() if (is_gps or i % 16 in ACT_ON_STG) else st_tile()
                for nq in range(2):
                    nc.tensor.matmul(
                        st[:, nq, :],
                        kT_h[:, blk * P:(blk + 1) * P],
                        qT_h[:, nq * 512:(nq + 1) * 512],
                        start=True,
                        stop=True,
                    )
                st_flat = st.rearrange("p a b -> p (a b)")
                pT = wpool.tile([P, QC], BF16, tag="pT", bufs=PT_BUFS)
                if is_gps:
                    stf = wpool.tile([P, QC], F32, tag="stf", bufs=STF_BUFS)
                    nc.vector.tensor_copy(stf, st_flat)
                    nc.gpsimd.tensor_tensor(
                        out=pT, in0=cbase.broadcast_to([P, QC]), in1=stf, op=POW,
                    )
                else:
                    nc.scalar.activation(pT, st_flat, EXP, scale=float(SCALE))
                fifo.append((i, qc, h, blk, pT))
                if len(fifo) > DELAY:
                    emit_pv(*fifo.pop(0))
            while fifo:
                emit_pv(*fifo.pop(0))

        # ---------------- output projection ----------------
        with tc.tile_pool(name="ps_f", bufs=1, space="PSUM") as ps_f:
            def final_transposes():
                # final pair (qc1, hp1): PE transposes through PSUM pipeline
                # with the outproj, instead of the serial DMA-transpose chain.
                # Emitted AFTER the qc0 chunks: they are gated by the last
                # normalize and would block ungated work on the in-order PE.
                for s in range(NSUB):
                    tp = ps_f.tile([P, P], BF16, tag="tp", bufs=2)
                    nc.tensor.transpose(tp, y_qc[1][:, 1, s, :], ident)
                    nc.vector.tensor_copy(
                        yT[:, 1, QC + s * P:QC + (s + 1) * P], tp)

            ob = bpool.tile([P, S // P, D], BF16, name="ob")
            for nb in range(S // P):
                if nb == 8:
                    final_transposes()
                pf = ps_f.tile([P, D], F32, tag="pf", bufs=4)
                for c in range(2):
                    nc.tensor.matmul(
                        pf,
                        yT[:, c, nb * P:(nb + 1) * P],
                        wo_sb[:, c, :],
                        start=(c == 0),
                        stop=(c == 1),
                    )
                if SPLIT_LAST_OB and nb >= 14:
                    # final chunks: halve the staging copy across both engines
                    nc.vector.tensor_copy(ob[:, nb, 0:D // 2], pf[:, 0:D // 2])
                    nc.scalar.copy(ob[:, nb, D // 2:D], pf[:, D // 2:D])
                elif nb % 2 == OB_DVE_PARITY:
                    nc.vector.tensor_copy(ob[:, nb, :], pf)
                else:
                    nc.scalar.copy(ob[:, nb, :], pf)
                if nb == 12:
                    nc.scalar.dma_start(out_d[12 * P:13 * P, :], ob[:, 12, :])
                elif nb == 13:
                    nc.sync.dma_start(out_d[13 * P:14 * P, :], ob[:, 13, :])
                elif nb == 14:
                    nc.sync.dma_start(out_d[14 * P:15 * P, :], ob[:, 14, :])
                elif nb == 15:
                    nc.scalar.dma_start(out_d[15 * P:16 * P, :], ob[:, 15, :])
                elif nb % 4 == 3:
                    g = nb // 4
                    eng = nc.scalar if g % 2 == 0 else nc.sync
                    eng.dma_start(
                        out_d[g * 4 * P:(g + 1) * 4 * P, :].rearrange(
                            "(j p) n -> p j n", p=P),
                        ob[:, g * 4:(g + 1) * 4, :],
                    )


_CACHED_NC = None


def _get_nc():
    global _CACHED_NC
    if _CACHED_NC is not None:
        return _CACHED_NC
    nc = bacc.Bacc("TRN2", target_bir_lowering=False, debug=False)
    xqT = nc.dram_tensor("xqT", [D, S], BF16, kind="ExternalInput").ap()
    xkT = nc.dram_tensor("xkT", [D, S], BF16, kind="ExternalInput").ap()
    xvT = nc.dram_tensor("xvT", [D, S], BF16, kind="ExternalInput").ap()
    wq = nc.dram_tensor("wq", [D, 2 * P], BF16, kind="ExternalInput").ap()
    wk = nc.dram_tensor("wk", [D, 2 * P], BF16, kind="ExternalInput").ap()
    wv = nc.dram_tensor("wv", [D, 2 * P], BF16, kind="ExternalInput").ap()
    wo = nc.dram_tensor("wo", [2 * P, D], BF16, kind="ExternalInput").ap()
    ident = nc.dram_tensor("ident", [P, P], BF16, kind="ExternalInput").ap()
    out = nc.dram_tensor("out", [S, D], BF16, kind="ExternalOutput").ap()
    with tile.TileContext(nc) as tc:
        _build_mha(tc, out, xqT, xkT, xvT, wq, wk, wv, wo, ident)
    nc.compile()
    _CACHED_NC = nc
    return nc


def _run(in_query, in_key, in_value, W_q, W_k, W_v, W_out, **run_kwargs):
    import ml_dtypes

    bf = lambda a: np.ascontiguousarray(np.asarray(a, dtype=np.float32)).astype(
        ml_dtypes.bfloat16)
    xqT = [bf(np.asarray(in_query[b], dtype=np.float32).T) for b in range(B)]
    xkT = [bf(np.asarray(in_key[b], dtype=np.float32).T) for b in range(B)]
    xvT = [bf(np.asarray(in_value[b], dtype=np.float32).T) for b in range(B)]
    W_q, W_k, W_v, W_out = (np.asarray(w, dtype=np.float32)
                            for w in (W_q, W_k, W_v, W_out))
    ident = np.eye(P, dtype=np.float32).astype(ml_dtypes.bfloat16)
    in_maps = []
    for c in range(NCORES):
        b, hh = c // 2, c % 2
        in_maps.append(
            {
                "xqT": xqT[b],
                "xkT": xkT[b],
                "xvT": xvT[b],
                "wq": bf(W_q[:, hh * 256:(hh + 1) * 256]),
                "wk": bf(W_k[:, hh * 256:(hh + 1) * 256]),
                "wv": bf(W_v[:, hh * 256:(hh + 1) * 256]),
                "wo": bf(W_out[hh * 256:(hh + 1) * 256, :]),
                "ident": ident,
            }
        )
    res = run_bass_kernel_spmd(_get_nc(), in_maps, list(range(NCORES)), **run_kwargs)
    out = np.empty((B, S, D), np.float32)
    for b in range(B):
        out[b] = (res.results[2 * b]["out"].astype(np.float32)
                  + res.results[2 * b + 1]["out"].astype(np.float32))
    return out, res


def kernel(in_query, in_key, in_value, W_q, W_k, W_v, W_out):
    out, _ = _run(in_query, in_key, in_value, W_q, W_k, W_v, W_out)
    return out
